# revision 4
# baseline (speedup 1.0000x reference)
"""CascadeHadamardLinear Trainium2 kernel.

Math (per token row x of length 4096):
  x_rot = (x * S_in) @ blockdiag(H_128)          (fp32, exact)
  x_q   = NVFP4 fake-quant of x_rot (16-elem blocks, e2m1 levels, RNE snap)
  out   = x_q @ W^T + (x_rot @ la^T) @ lb^T + bias

Distribution: pure data-parallel over the 8192 token rows (1024/core, 8 cores).

Device pipeline per core (tokens NT=1024):
  inputs are host-pre-transposed: xT [4096, 1024], wT [4096, 4096],
  laT [4096, 32], lbT [32, 4096].
  Phase 1 (per 4-block group jg of the 32 Hadamard blocks):
    rotation MMs (lhsT = xT block, rhs = S-folded H) -> PSUM [128t, 512]
    quant chain on each bank: absmax-16 reduce, z = x*6/amax,
    t2 = RNE_int(z), custom DVE ops SNAP_SEL23/SNAP_SELF -> e2m1 level f,
    xq = f * (amax/6) in bf16;  LoRA1 accumulates t1T = la_eff^T @ x.
    PE transposes xq -> xqT (feature-major, bf16).
  Phase 2: out[t, o] = bias (K=1 MM) + sum_d xqT^T @ wT_bf + t1T^T @ lbT (PSUM
    accumulation), evacuate fp32, DMA out.
"""

import os
import sys

for _p in ("/opt/trn_rl_repo",):
    if os.path.isdir(_p) and _p not in sys.path:
        sys.path.insert(0, _p)

import numpy as np

import concourse.bass as bass
import concourse.mybir as mybir
import concourse.tile as tile
from concourse import bacc
from concourse.bass_utils import run_bass_kernel_spmd

F32 = np.float32

# ---------------- problem constants (hardcoded per contract) ----------------
B, S, D_IN, D_OUT, RANK, HBS = 4, 2048, 4096, 4096, 32, 128
NTOK = B * S                  # 8192
NCORES = 8
NT = NTOK // NCORES           # 1024 tokens per core
NJ = D_IN // HBS              # 32 hadamard blocks
QB = 16                       # quant block size
TS_N = NT // 128              # 8 token sub-tiles of 128
OG_N = D_OUT // 512           # 8 output column groups

# quant snap constants (1.5*2^k magic so ulp is uniform on both sides of c)
C_INT = 12582912.0            # 1.5*2^23, ulp 1.0
C_HALF = 6291456.0            # 1.5*2^22, ulp 0.5
TH23 = 20.25                  # 4.5^2
THF = 5.0625                  # 2.25^2

# ---------------- custom DVE ops (e2m1 level snap) ----------------
def _register_snap_ops():
    from concourse.dve_spec import (
        Spec, Src0, Src1, C0, C1, lower as dve_lower, sq, select, _has_src1,
    )
    from concourse.dve_ops import (
        DveOp, OPS, CUSTOM_DVE_SPECS, _SUB_OPCODE_FOR_NAME, _CUSTOM_DVE_ROW_BASE,
    )
    from concourse.dve_uop import DveOpSpec
    from concourse.dve_table_gen import dve_ver_for

    def _ref_sel23(in0, in1, c0, c1, c2):
        z = in0.astype(F32)
        c3 = F32(2.0) * F32(c1)
        t3 = (z + c3) - c3
        return np.where(z * z < F32(c0), in1.astype(F32), t3).astype(F32)

    def _ref_self(in0, in1, c0, c1, c2):
        z = in0.astype(F32)
        c1 = F32(c1)
        t1 = (z + c1) - c1
        return np.where(z * z < F32(c0), t1, in1.astype(F32)).astype(F32)

    def _mk(name, body, ref):
        if name in _SUB_OPCODE_FOR_NAME:
            return next(op for op in OPS if op.name == name)
        spec = Spec(body=body, reference=ref)
        row = _CUSTOM_DVE_ROW_BASE + len(OPS)
        assert row < 0x20
        ver = dve_ver_for("TRN2")
        uops = dve_lower(spec, ver=ver)
        sha = DveOpSpec(
            name=name, opcode=row, uops=uops, rd1_en=_has_src1(spec)
        ).sha(ver)
        op = DveOp(name, spec, subdim=False, uops_sha={ver: sha})
        OPS.append(op)
        CUSTOM_DVE_SPECS[name] = spec
        _SUB_OPCODE_FOR_NAME[name] = row
        return op

    z = Src0
    c3 = C1 + C1
    sel23 = _mk(
        "SNAP_SEL23_ANT",
        select(sq(z) < C0, Src1, (z + c3) - c3),
        _ref_sel23,
    )
    self_ = _mk(
        "SNAP_SELF_ANT",
        select(sq(z) < C0, (z + C1) - C1, Src1),
        _ref_self,
    )
    return sel23, self_


SNAP_SEL23, SNAP_SELF = _register_snap_ops()


# ---------------- device kernel ----------------
def _build_nc():
    nc = bacc.Bacc(
        "TRN2", target_bir_lowering=False, debug=False, num_devices=NCORES
    )
    dt = mybir.dt
    xT = nc.dram_tensor("xT", [D_IN, NT], dt.float32, kind="ExternalInput")
    wT = nc.dram_tensor("wT", [D_IN, D_OUT], dt.float32, kind="ExternalInput")
    H = nc.dram_tensor("H", [HBS, HBS], dt.float32, kind="ExternalInput")
    Scol = nc.dram_tensor("Scol", [HBS, NJ], dt.float32, kind="ExternalInput")
    laT = nc.dram_tensor("laT", [D_IN, RANK], dt.float32, kind="ExternalInput")
    lbT = nc.dram_tensor("lbT", [RANK, D_OUT], dt.float32, kind="ExternalInput")
    bias = nc.dram_tensor("bias", [1, D_OUT], dt.float32, kind="ExternalInput")
    y = nc.dram_tensor("y", [NT, D_OUT], dt.float32, kind="ExternalOutput")

    with tile.TileContext(nc) as tc:
        _emit(nc, tc, xT, wT, H, Scol, laT, lbT, bias, y)
    nc.compile()
    return nc


def _emit(nc, tc, xT, wT, H, Scol, laT, lbT, bias, y):
    from contextlib import ExitStack

    dt = mybir.dt
    Alu = mybir.AluOpType
    Act = mybir.ActivationFunctionType

    with ExitStack() as ctx:
        # ---- persistent pools ----
        singles = ctx.enter_context(tc.tile_pool(name="singles", bufs=1))
        xqT_pool = ctx.enter_context(tc.tile_pool(name="xqT", bufs=1))

        # persistent small constants (phase 2 + cross-phase)
        lbT_bf = singles.tile([RANK, D_OUT], dt.bfloat16)
        bias_bf = singles.tile([1, D_OUT], dt.bfloat16)
        ones_bf = singles.tile([1, 128], dt.bfloat16)
        nc.vector.memset(ones_bf[:], 1.0)
        with tc.tile_pool(name="setup_tmp", bufs=1) as stp:
            lbT_f32 = stp.tile([RANK, D_OUT], dt.float32)
            nc.sync.dma_start(out=lbT_f32[:], in_=lbT[:])
            nc.gpsimd.tensor_copy(out=lbT_bf[:], in_=lbT_f32[:])
            bias_f32 = stp.tile([1, D_OUT], dt.float32)
            nc.sync.dma_start(out=bias_f32[:], in_=bias[:])
            nc.gpsimd.tensor_copy(out=bias_bf[:], in_=bias_f32[:])

        # xqT[c, dblk, t] : feature-major quantized activations (bf16)
        xqT = xqT_pool.tile([128, NJ, NT], dt.bfloat16)
        # t1T[r, t] : LoRA1 result, bf16
        t1T_bf = singles.tile([RANK, NT], dt.bfloat16)

        # ---- phase 1: rotation + quant + LoRA1 + xq transpose ----
        with ExitStack() as p1:
            consts1 = p1.enter_context(tc.tile_pool(name="consts1", bufs=1))
            H_sb = consts1.tile([HBS, HBS], dt.float32)
            nc.sync.dma_start(out=H_sb[:], in_=H[:])
            S_sb = consts1.tile([HBS, NJ], dt.float32)
            nc.sync.dma_start(out=S_sb[:], in_=Scol[:])
            laT_sb = consts1.tile([HBS, NJ, RANK], dt.float32)
            nc.sync.dma_start(
                out=laT_sb[:], in_=laT[:].rearrange("(j c) r -> c j r", c=HBS)
            )

            # H_mod[c, j, c'] = S[j*128+c] * H[c, c']   (rotation rhs, fp32)
            H_mod = consts1.tile([HBS, NJ, HBS], dt.float32)
            for j in range(NJ):
                nc.vector.tensor_scalar(
                    out=H_mod[:, j, :], in0=H_sb[:], scalar1=S_sb[:, j : j + 1],
                    scalar2=None, op0=Alu.mult,
                )

            # la_eff[c, j, r] = S[j*128+c] * (H @ laT_j)[c, r]
            la_eff = consts1.tile([HBS, NJ, RANK], dt.float32)
            with tc.tile_pool(name="psla", bufs=2, space="PSUM") as psla_pool:
                for j in range(NJ):
                    ps = psla_pool.tile([HBS, RANK], dt.float32)
                    nc.tensor.matmul(ps[:], lhsT=H_sb[:], rhs=laT_sb[:, j, :])
                    nc.scalar.activation(
                        out=la_eff[:, j, :], in_=ps[:], func=Act.Copy,
                        scale=S_sb[:, j : j + 1],
                    )

            # identity for bf16 PE transposes
            ident_bf = consts1.tile([128, 128], dt.bfloat16)
            from concourse.masks import make_identity
            make_identity(nc, ident_bf[:])

            xt_pool = p1.enter_context(tc.tile_pool(name="xt", bufs=6))
            qtmp = p1.enter_context(tc.tile_pool(name="qtmp", bufs=3))
            qsm = p1.enter_context(tc.tile_pool(name="qsm", bufs=4))
            xq_pool = p1.enter_context(tc.tile_pool(name="xq", bufs=10))
            rot_ps = p1.enter_context(
                tc.tile_pool(name="rotps", bufs=3, space="PSUM")
            )
            tr_ps = p1.enter_context(
                tc.tile_pool(name="trps", bufs=2, space="PSUM")
            )
            t1_ps = p1.enter_context(
                tc.tile_pool(name="t1ps", bufs=1, space="PSUM")
            )

            t1_acc = [
                t1_ps.tile([RANK, 512], dt.float32, name=f"t1acc{h}")
                for h in range(2)
            ]

            for jg in range(NJ // 4):
                # load the 4 xT blocks of this group; LoRA1 MMs
                xts = []
                for dj in range(4):
                    j = 4 * jg + dj
                    xt = xt_pool.tile([HBS, NT], dt.float32)
                    nc.sync.dma_start(
                        out=xt[:], in_=xT[j * HBS : (j + 1) * HBS, :]
                    )
                    xts.append(xt)
                    for h in range(2):
                        nc.tensor.matmul(
                            t1_acc[h][:],
                            lhsT=la_eff[:, j, :],
                            rhs=xt[:, h * 512 : (h + 1) * 512],
                            start=(j == 0),
                            stop=(j == NJ - 1),
                        )

                xq_tiles = []
                for ts in range(TS_N):
                    # rotation: 4 blocks -> one PSUM bank [128t, 512c']
                    bank = rot_ps.tile([128, 512], dt.float32)
                    for dj in range(4):
                        j = 4 * jg + dj
                        nc.tensor.matmul(
                            bank[:, dj * HBS : (dj + 1) * HBS],
                            lhsT=xts[dj][:, ts * 128 : (ts + 1) * 128],
                            rhs=H_mod[:, j, :],
                            start=(dj == 0),
                            stop=(dj == 3),
                        )
                    # quant chain on this bank
                    nb = 512 // QB  # 32 16-elem blocks
                    amax = qsm.tile([128, nb], dt.float32)
                    nc.vector.tensor_reduce(
                        out=amax[:],
                        in_=bank[:].rearrange("p (b s) -> p b s", s=QB),
                        axis=mybir.AxisListType.X,
                        op=Alu.max,
                        apply_absolute_value=True,
                    )
                    ra = qsm.tile([128, nb], dt.float32)
                    nc.vector.reciprocal(out=ra[:], in_=amax[:])
                    rs6 = qsm.tile([128, nb], dt.float32)
                    nc.scalar.mul(out=rs6[:], in_=ra[:], mul=6.0)
                    sc = qsm.tile([128, nb], dt.float32)
                    nc.scalar.mul(out=sc[:], in_=amax[:], mul=1.0 / 6.0)
                    z = qtmp.tile([128, 512], dt.float32)
                    nc.vector.tensor_tensor(
                        out=z[:].rearrange("p (b s) -> p b s", s=QB),
                        in0=bank[:].rearrange("p (b s) -> p b s", s=QB),
                        in1=rs6[:].unsqueeze(2).broadcast_to([128, nb, QB]),
                        op=Alu.mult,
                    )
                    t2 = qtmp.tile([128, 512], dt.float32)
                    nc.vector.tensor_scalar(
                        out=t2[:], in0=z[:], scalar1=C_INT, scalar2=C_INT,
                        op0=Alu.add, op1=Alu.subtract,
                    )
                    r23 = qtmp.tile([128, 512], dt.float32)
                    nc.vector._custom_dve(
                        SNAP_SEL23, out=r23[:], in0=z[:], in1=t2[:],
                        s0=TH23, s1=C_INT,
                    )
                    f = qtmp.tile([128, 512], dt.float32)
                    nc.vector._custom_dve(
                        SNAP_SELF, out=f[:], in0=z[:], in1=r23[:],
                        s0=THF, s1=C_HALF,
                    )
                    xq_t = xq_pool.tile([128, 512], dt.bfloat16)
                    nc.vector.tensor_tensor(
                        out=xq_t[:].rearrange("p (b s) -> p b s", s=QB),
                        in0=f[:].rearrange("p (b s) -> p b s", s=QB),
                        in1=sc[:].unsqueeze(2).broadcast_to([128, nb, QB]),
                        op=Alu.mult,
                    )
                    xq_tiles.append(xq_t)

                # transpose xq -> xqT for the 4 feature blocks of this group
                for dj in range(4):
                    j = 4 * jg + dj
                    pt = tr_ps.tile([128, NT], dt.bfloat16)
                    for ts in range(TS_N):
                        nc.tensor.matmul(
                            pt[:, ts * 128 : (ts + 1) * 128],
                            lhsT=xq_tiles[ts][:, dj * HBS : (dj + 1) * HBS],
                            rhs=ident_bf[:],
                            is_transpose=True,
                            start=(ts == 0),
                            stop=(ts == TS_N - 1),
                        )
                    nc.scalar.copy(out=xqT[:, j, :], in_=pt[:])

            # evacuate LoRA1 accumulators
            for h in range(2):
                nc.scalar.copy(
                    out=t1T_bf[:, h * 512 : (h + 1) * 512], in_=t1_acc[h][:]
                )

        # ---- phase 2: main GEMM + bias + LoRA2 ----
        with ExitStack() as p2:
            wst_pool = p2.enter_context(tc.tile_pool(name="wst", bufs=2))
            wbf_pool = p2.enter_context(tc.tile_pool(name="wbf", bufs=1))
            out_pool = p2.enter_context(tc.tile_pool(name="out", bufs=4))
            out_ps = p2.enter_context(
                tc.tile_pool(name="outps", bufs=4, space="PSUM")
            )

            for og in range(OG_N):
                osl = slice(og * 512, (og + 1) * 512)
                wbf = wbf_pool.tile([128, NJ, 512], dt.bfloat16)
                for wc in range(4):  # 8 dblks per stage chunk
                    wst = wst_pool.tile([128, 8, 512], dt.float32)
                    nc.sync.dma_start(
                        out=wst[:],
                        in_=wT[:, osl].rearrange(
                            "(j c) o -> c j o", c=HBS
                        )[:, wc * 8 : (wc + 1) * 8, :],
                    )
                    nc.gpsimd.tensor_copy(
                        out=wbf[:, wc * 8 : (wc + 1) * 8, :], in_=wst[:]
                    )
                for ts in range(TS_N):
                    tsl = slice(ts * 128, (ts + 1) * 128)
                    po = out_ps.tile([128, 512], dt.float32)
                    nc.tensor.matmul(
                        po[:], lhsT=ones_bf[:], rhs=bias_bf[:, osl],
                        start=True, stop=False,
                    )
                    for dblk in range(NJ):
                        nc.tensor.matmul(
                            po[:],
                            lhsT=xqT[:, dblk, tsl],
                            rhs=wbf[:, dblk, :],
                            start=False,
                            stop=False,
                        )
                    nc.tensor.matmul(
                        po[:], lhsT=t1T_bf[:, tsl], rhs=lbT_bf[:, osl],
                        start=False, stop=True,
                    )
                    ot = out_pool.tile([128, 512], dt.float32)
                    nc.scalar.copy(out=ot[:], in_=po[:])
                    nc.sync.dma_start(out=y[tsl, osl], in_=ot[:])


_NC_CACHE = None


def _get_nc():
    global _NC_CACHE
    if _NC_CACHE is None:
        _NC_CACHE = _build_nc()
    return _NC_CACHE


# ---------------- host wrapper ----------------
def kernel(x, S_in, H_block, w_quantized, lora_a, lora_b, bias):
    x = np.asarray(x, dtype=F32)
    S_in = np.asarray(S_in, dtype=F32)
    H_block = np.ascontiguousarray(np.asarray(H_block, dtype=F32))
    w_quantized = np.asarray(w_quantized, dtype=F32)
    lora_a = np.asarray(lora_a, dtype=F32)
    lora_b = np.asarray(lora_b, dtype=F32)
    bias = np.asarray(bias, dtype=F32)

    x_flat = x.reshape(NTOK, D_IN)
    wT = np.ascontiguousarray(w_quantized.T)            # [D_IN, D_OUT]
    laT = np.ascontiguousarray(lora_a.T)                # [D_IN, RANK]
    lbT = np.ascontiguousarray(lora_b.T)                # [RANK, D_OUT]
    Scol = np.ascontiguousarray(S_in.reshape(NJ, HBS).T)  # [HBS, NJ]
    bias2d = np.ascontiguousarray(bias.reshape(1, D_OUT))

    nc = _get_nc()
    in_maps = []
    for c in range(NCORES):
        xT_c = np.ascontiguousarray(x_flat[c * NT : (c + 1) * NT].T)
        in_maps.append(
            {
                "xT": xT_c,
                "wT": wT,
                "H": H_block,
                "Scol": Scol,
                "laT": laT,
                "lbT": lbT,
                "bias": bias2d,
            }
        )
    res = run_bass_kernel_spmd(nc, in_maps, core_ids=list(range(NCORES)))
    out = np.concatenate([res.results[c]["y"] for c in range(NCORES)], axis=0)
    return out.reshape(B, S, D_OUT).astype(F32)


# revision 6
# speedup vs baseline: 1.4027x; 1.4027x over previous
"""CascadeHadamardLinear Trainium2 kernel.

Math (per token row x of length 4096):
  x_rot = (x * S_in) @ blockdiag(H_128)          (fp32, exact)
  x_q   = NVFP4 fake-quant of x_rot (16-elem blocks, e2m1 levels, RNE snap)
  out   = x_q @ W^T + (x_rot @ la^T) @ lb^T + bias

Distribution: pure data-parallel over the 8192 token rows (1024/core, 8 cores).

Device pipeline per core (tokens NT=1024):
  inputs are host-pre-transposed: xT [4096, 1024], wT [4096, 4096],
  laT [4096, 32], lbT [32, 4096].
  Phase 1 (per 4-block group jg of the 32 Hadamard blocks):
    rotation MMs (lhsT = xT block, rhs = S-folded H) -> PSUM [128t, 512]
    quant chain on each bank: absmax-16 reduce, z = x*6/amax,
    t2 = RNE_int(z), custom DVE ops SNAP_SEL23/SNAP_SELF -> e2m1 level f,
    xq = f * (amax/6) in bf16;  LoRA1 accumulates t1T = la_eff^T @ x.
    PE transposes xq -> xqT (feature-major, bf16).
  Phase 2: out[t, o] = bias (K=1 MM) + sum_d xqT^T @ wT_bf + t1T^T @ lbT (PSUM
    accumulation), evacuate fp32, DMA out.
"""

import os
import sys

for _p in ("/opt/trn_rl_repo",):
    if os.path.isdir(_p) and _p not in sys.path:
        sys.path.insert(0, _p)

import numpy as np

import concourse.bass as bass
import concourse.mybir as mybir
import concourse.tile as tile
from concourse import bacc
from concourse.bass_utils import run_bass_kernel_spmd

F32 = np.float32

# ---------------- problem constants (hardcoded per contract) ----------------
B, S, D_IN, D_OUT, RANK, HBS = 4, 2048, 4096, 4096, 32, 128
NTOK = B * S                  # 8192
NCORES = 8
NT = NTOK // NCORES           # 1024 tokens per core
NJ = D_IN // HBS              # 32 hadamard blocks
QB = 16                       # quant block size
TS_N = NT // 128              # 8 token sub-tiles of 128
OG_N = D_OUT // 512           # 8 output column groups

# quant snap constants (1.5*2^k magic so ulp is uniform on both sides of c)
C_INT = 12582912.0            # 1.5*2^23, ulp 1.0
C_HALF = 6291456.0            # 1.5*2^22, ulp 0.5
TH23 = 20.25                  # 4.5^2
THF = 5.0625                  # 2.25^2

# ---------------- custom DVE ops (e2m1 level snap) ----------------
def _register_snap_ops():
    from concourse.dve_spec import (
        Spec, Src0, Src1, C0, C1, lower as dve_lower, sq, select, _has_src1,
    )
    from concourse.dve_ops import (
        DveOp, OPS, CUSTOM_DVE_SPECS, _SUB_OPCODE_FOR_NAME, _CUSTOM_DVE_ROW_BASE,
    )
    from concourse.dve_uop import DveOpSpec
    from concourse.dve_table_gen import dve_ver_for

    def _ref_sel23(in0, in1, c0, c1, c2):
        z = in0.astype(F32)
        c3 = F32(2.0) * F32(c1)
        t3 = (z + c3) - c3
        return np.where(z * z < F32(c0), in1.astype(F32), t3).astype(F32)

    def _ref_self(in0, in1, c0, c1, c2):
        z = in0.astype(F32)
        c1 = F32(c1)
        t1 = (z + c1) - c1
        return np.where(z * z < F32(c0), t1, in1.astype(F32)).astype(F32)

    def _mk(name, body, ref):
        if name in _SUB_OPCODE_FOR_NAME:
            return next(op for op in OPS if op.name == name)
        spec = Spec(body=body, reference=ref)
        row = _CUSTOM_DVE_ROW_BASE + len(OPS)
        assert row < 0x20
        ver = dve_ver_for("TRN2")
        uops = dve_lower(spec, ver=ver)
        sha = DveOpSpec(
            name=name, opcode=row, uops=uops, rd1_en=_has_src1(spec)
        ).sha(ver)
        op = DveOp(name, spec, subdim=False, uops_sha={ver: sha})
        OPS.append(op)
        CUSTOM_DVE_SPECS[name] = spec
        _SUB_OPCODE_FOR_NAME[name] = row
        return op

    z = Src0
    c3 = C1 + C1
    sel23 = _mk(
        "SNAP_SEL23_ANT",
        select(sq(z) < C0, Src1, (z + c3) - c3),
        _ref_sel23,
    )
    self_ = _mk(
        "SNAP_SELF_ANT",
        select(sq(z) < C0, (z + C1) - C1, Src1),
        _ref_self,
    )
    return sel23, self_


SNAP_SEL23, SNAP_SELF = _register_snap_ops()


# ---------------- device kernel ----------------
def _build_nc():
    nc = bacc.Bacc(
        "TRN2", target_bir_lowering=False, debug=False, num_devices=NCORES
    )
    dt = mybir.dt
    xT = nc.dram_tensor("xT", [D_IN, NT], dt.float32, kind="ExternalInput")
    wT = nc.dram_tensor("wT", [D_IN, D_OUT], dt.bfloat16, kind="ExternalInput")
    H = nc.dram_tensor("H", [HBS, HBS], dt.float32, kind="ExternalInput")
    Scol = nc.dram_tensor("Scol", [HBS, NJ], dt.float32, kind="ExternalInput")
    laT = nc.dram_tensor("laT", [D_IN, RANK], dt.float32, kind="ExternalInput")
    lbT = nc.dram_tensor("lbT", [RANK, D_OUT], dt.bfloat16, kind="ExternalInput")
    bias = nc.dram_tensor("bias", [1, D_OUT], dt.bfloat16, kind="ExternalInput")
    y = nc.dram_tensor("y", [NT, D_OUT], dt.float32, kind="ExternalOutput")

    with tile.TileContext(nc) as tc:
        _emit(nc, tc, xT, wT, H, Scol, laT, lbT, bias, y)
    nc.compile()
    return nc


def _emit(nc, tc, xT, wT, H, Scol, laT, lbT, bias, y):
    from contextlib import ExitStack

    dt = mybir.dt
    Alu = mybir.AluOpType
    Act = mybir.ActivationFunctionType

    with ExitStack() as ctx:
        # ---- persistent pools ----
        singles = ctx.enter_context(tc.tile_pool(name="singles", bufs=1))
        xqT_pool = ctx.enter_context(tc.tile_pool(name="xqT", bufs=1))

        # persistent small constants (phase 2 + cross-phase)
        lbT_bf = singles.tile([RANK, D_OUT], dt.bfloat16)
        nc.sync.dma_start(out=lbT_bf[:], in_=lbT[:])
        bias_bf = singles.tile([1, D_OUT], dt.bfloat16)
        nc.sync.dma_start(out=bias_bf[:], in_=bias[:])
        ones_bf = singles.tile([1, 128], dt.bfloat16)
        nc.vector.memset(ones_bf[:], 1.0)

        # xqT[c, dblk, t] : feature-major quantized activations (bf16)
        xqT = xqT_pool.tile([128, NJ, NT], dt.bfloat16)
        # t1T[r, t] : LoRA1 result, bf16
        t1T_bf = singles.tile([RANK, NT], dt.bfloat16)

        # ---- phase 1: rotation + quant + LoRA1 + xq transpose ----
        with ExitStack() as p1:
            consts1 = p1.enter_context(tc.tile_pool(name="consts1", bufs=1))
            H_sb = consts1.tile([HBS, HBS], dt.float32)
            nc.sync.dma_start(out=H_sb[:], in_=H[:])
            S_sb = consts1.tile([HBS, NJ], dt.float32)
            nc.sync.dma_start(out=S_sb[:], in_=Scol[:])
            laT_sb = consts1.tile([HBS, NJ, RANK], dt.float32)
            nc.sync.dma_start(
                out=laT_sb[:], in_=laT[:].rearrange("(j c) r -> c j r", c=HBS)
            )

            # H_mod[c, j, c'] = S[j*128+c] * H[c, c']   (rotation rhs, fp32)
            H_mod = consts1.tile([HBS, NJ, HBS], dt.float32)
            for j in range(NJ):
                nc.vector.tensor_scalar(
                    out=H_mod[:, j, :], in0=H_sb[:], scalar1=S_sb[:, j : j + 1],
                    scalar2=None, op0=Alu.mult,
                )

            # la_eff[c, j, r] = S[j*128+c] * (H @ laT_j)[c, r]
            la_eff = consts1.tile([HBS, NJ, RANK], dt.float32)
            with tc.tile_pool(name="psla", bufs=2, space="PSUM") as psla_pool:
                for j in range(NJ):
                    ps = psla_pool.tile([HBS, RANK], dt.float32)
                    nc.tensor.matmul(ps[:], lhsT=H_sb[:], rhs=laT_sb[:, j, :])
                    nc.scalar.activation(
                        out=la_eff[:, j, :], in_=ps[:], func=Act.Copy,
                        scale=S_sb[:, j : j + 1],
                    )

            # identity for bf16 PE transposes
            ident_bf = consts1.tile([128, 128], dt.bfloat16)
            from concourse.masks import make_identity
            make_identity(nc, ident_bf[:])

            xt_pool = p1.enter_context(tc.tile_pool(name="xt", bufs=6))
            qtmp = p1.enter_context(tc.tile_pool(name="qtmp", bufs=3))
            qsm = p1.enter_context(tc.tile_pool(name="qsm", bufs=4))
            xq_pool = p1.enter_context(tc.tile_pool(name="xq", bufs=10))
            rot_ps = p1.enter_context(
                tc.tile_pool(name="rotps", bufs=3, space="PSUM")
            )
            tr_ps = p1.enter_context(
                tc.tile_pool(name="trps", bufs=2, space="PSUM")
            )
            t1_ps = p1.enter_context(
                tc.tile_pool(name="t1ps", bufs=1, space="PSUM")
            )

            t1_acc = [
                t1_ps.tile([RANK, 512], dt.float32, name=f"t1acc{h}")
                for h in range(2)
            ]

            for jg in range(NJ // 4):
                # load the 4 xT blocks of this group; LoRA1 MMs
                xts = []
                for dj in range(4):
                    j = 4 * jg + dj
                    xt = xt_pool.tile([HBS, NT], dt.float32)
                    nc.sync.dma_start(
                        out=xt[:], in_=xT[j * HBS : (j + 1) * HBS, :]
                    )
                    xts.append(xt)
                    for h in range(2):
                        nc.tensor.matmul(
                            t1_acc[h][:],
                            lhsT=la_eff[:, j, :],
                            rhs=xt[:, h * 512 : (h + 1) * 512],
                            start=(j == 0),
                            stop=(j == NJ - 1),
                        )

                xq_tiles = []
                for ts in range(TS_N):
                    # rotation: 4 blocks -> one PSUM bank [128t, 512c']
                    bank = rot_ps.tile([128, 512], dt.float32)
                    for dj in range(4):
                        j = 4 * jg + dj
                        nc.tensor.matmul(
                            bank[:, dj * HBS : (dj + 1) * HBS],
                            lhsT=xts[dj][:, ts * 128 : (ts + 1) * 128],
                            rhs=H_mod[:, j, :],
                            start=(dj == 0),
                            stop=(dj == 3),
                        )
                    # quant chain on this bank
                    nb = 512 // QB  # 32 16-elem blocks
                    amax = qsm.tile([128, nb], dt.float32)
                    nc.vector.tensor_reduce(
                        out=amax[:],
                        in_=bank[:].rearrange("p (b s) -> p b s", s=QB),
                        axis=mybir.AxisListType.X,
                        op=Alu.max,
                        apply_absolute_value=True,
                    )
                    ra = qsm.tile([128, nb], dt.float32)
                    nc.vector.reciprocal(out=ra[:], in_=amax[:])
                    rs6 = qsm.tile([128, nb], dt.float32)
                    nc.scalar.mul(out=rs6[:], in_=ra[:], mul=6.0)
                    sc = qsm.tile([128, nb], dt.float32)
                    nc.scalar.mul(out=sc[:], in_=amax[:], mul=1.0 / 6.0)
                    z = qtmp.tile([128, 512], dt.float32)
                    nc.vector.tensor_tensor(
                        out=z[:].rearrange("p (b s) -> p b s", s=QB),
                        in0=bank[:].rearrange("p (b s) -> p b s", s=QB),
                        in1=rs6[:].unsqueeze(2).broadcast_to([128, nb, QB]),
                        op=Alu.mult,
                    )
                    t2 = qtmp.tile([128, 512], dt.float32)
                    nc.vector.tensor_scalar(
                        out=t2[:], in0=z[:], scalar1=C_INT, scalar2=C_INT,
                        op0=Alu.add, op1=Alu.subtract,
                    )
                    r23 = qtmp.tile([128, 512], dt.float32)
                    nc.vector._custom_dve(
                        SNAP_SEL23, out=r23[:], in0=z[:], in1=t2[:],
                        s0=TH23, s1=C_INT,
                    )
                    f = qtmp.tile([128, 512], dt.float32)
                    nc.vector._custom_dve(
                        SNAP_SELF, out=f[:], in0=z[:], in1=r23[:],
                        s0=THF, s1=C_HALF,
                    )
                    xq_t = xq_pool.tile([128, 512], dt.bfloat16)
                    nc.vector.tensor_tensor(
                        out=xq_t[:].rearrange("p (b s) -> p b s", s=QB),
                        in0=f[:].rearrange("p (b s) -> p b s", s=QB),
                        in1=sc[:].unsqueeze(2).broadcast_to([128, nb, QB]),
                        op=Alu.mult,
                    )
                    xq_tiles.append(xq_t)

                # transpose xq -> xqT for the 4 feature blocks of this group
                for dj in range(4):
                    j = 4 * jg + dj
                    pt = tr_ps.tile([128, NT], dt.bfloat16)
                    for ts in range(TS_N):
                        nc.tensor.matmul(
                            pt[:, ts * 128 : (ts + 1) * 128],
                            lhsT=xq_tiles[ts][:, dj * HBS : (dj + 1) * HBS],
                            rhs=ident_bf[:],
                            is_transpose=True,
                            start=(ts == 0),
                            stop=(ts == TS_N - 1),
                        )
                    nc.scalar.copy(out=xqT[:, j, :], in_=pt[:])

            # evacuate LoRA1 accumulators
            for h in range(2):
                nc.scalar.copy(
                    out=t1T_bf[:, h * 512 : (h + 1) * 512], in_=t1_acc[h][:]
                )

        # ---- phase 2: main GEMM + bias + LoRA2 ----
        with ExitStack() as p2:
            wbf_pool = p2.enter_context(tc.tile_pool(name="wbf", bufs=2))
            out_pool = p2.enter_context(tc.tile_pool(name="out", bufs=4))
            out_ps = p2.enter_context(
                tc.tile_pool(name="outps", bufs=4, space="PSUM")
            )

            for og in range(OG_N):
                osl = slice(og * 512, (og + 1) * 512)
                wbf = wbf_pool.tile([128, NJ, 512], dt.bfloat16)
                nc.sync.dma_start(
                    out=wbf[:],
                    in_=wT[:, osl].rearrange("(j c) o -> c j o", c=HBS),
                )
                for ts in range(TS_N):
                    tsl = slice(ts * 128, (ts + 1) * 128)
                    po = out_ps.tile([128, 512], dt.float32)
                    nc.tensor.matmul(
                        po[:], lhsT=ones_bf[:], rhs=bias_bf[:, osl],
                        start=True, stop=False,
                    )
                    for dblk in range(NJ):
                        nc.tensor.matmul(
                            po[:],
                            lhsT=xqT[:, dblk, tsl],
                            rhs=wbf[:, dblk, :],
                            start=False,
                            stop=False,
                        )
                    nc.tensor.matmul(
                        po[:], lhsT=t1T_bf[:, tsl], rhs=lbT_bf[:, osl],
                        start=False, stop=True,
                    )
                    ot = out_pool.tile([128, 512], dt.float32)
                    nc.scalar.copy(out=ot[:], in_=po[:])
                    nc.sync.dma_start(out=y[tsl, osl], in_=ot[:])


_NC_CACHE = None


def _get_nc():
    global _NC_CACHE
    if _NC_CACHE is None:
        _NC_CACHE = _build_nc()
    return _NC_CACHE


# ---------------- host wrapper ----------------
def kernel(x, S_in, H_block, w_quantized, lora_a, lora_b, bias):
    x = np.asarray(x, dtype=F32)
    S_in = np.asarray(S_in, dtype=F32)
    H_block = np.ascontiguousarray(np.asarray(H_block, dtype=F32))
    w_quantized = np.asarray(w_quantized, dtype=F32)
    lora_a = np.asarray(lora_a, dtype=F32)
    lora_b = np.asarray(lora_b, dtype=F32)
    bias = np.asarray(bias, dtype=F32)

    import ml_dtypes
    BF16 = ml_dtypes.bfloat16
    x_flat = x.reshape(NTOK, D_IN)
    wT = np.ascontiguousarray(w_quantized.T.astype(BF16))   # [D_IN, D_OUT] bf16
    laT = np.ascontiguousarray(lora_a.T)                    # [D_IN, RANK]
    lbT = np.ascontiguousarray(lora_b.T.astype(BF16))       # [RANK, D_OUT] bf16
    Scol = np.ascontiguousarray(S_in.reshape(NJ, HBS).T)    # [HBS, NJ]
    bias2d = np.ascontiguousarray(bias.reshape(1, D_OUT).astype(BF16))

    nc = _get_nc()
    in_maps = []
    for c in range(NCORES):
        xT_c = np.ascontiguousarray(x_flat[c * NT : (c + 1) * NT].T)
        in_maps.append(
            {
                "xT": xT_c,
                "wT": wT,
                "H": H_block,
                "Scol": Scol,
                "laT": laT,
                "lbT": lbT,
                "bias": bias2d,
            }
        )
    res = run_bass_kernel_spmd(nc, in_maps, core_ids=list(range(NCORES)))
    out = np.concatenate([res.results[c]["y"] for c in range(NCORES)], axis=0)
    return out.reshape(B, S, D_OUT).astype(F32)
